# revision 1
# baseline (speedup 1.0000x reference)
"""Mixtral decoder layer (attention + top-2-of-8 MoE) on 8 trn2 NeuronCores.

Sharding: sequence-parallel attention (each core owns 256 query rows),
AllGather of post-attention hidden states, expert-parallel MoE (one expert
per core, weights fed per-core) with ReduceScatter combine. Host only
slices inputs / concatenates disjoint output shards.
"""

import numpy as np

import concourse.bass as bass
import concourse.mybir as mybir
import concourse.tile as tile
from concourse.vector_clock import ScopedClock
from concourse.masks import make_identity
from concourse.bass_utils import run_bass_kernel_spmd

# ---------------------------------------------------------------- constants
NCORES = 8
B, S, H = 1, 2048, 2048
NH, NKV, HD = 16, 4, 128
F, E = 4096, 8
EPS = 1e-5
THETA = 10000.0
NEG = -1e30
QR = S // NCORES          # query rows per core = 256
P = 128
F32 = mybir.dt.float32
F32R = mybir.dt.float32r
AX = mybir.AxisListType.X
ALU = mybir.AluOpType
ACT = mybir.ActivationFunctionType

TT = S // P        # 16 token tiles
HT = H // P        # 16 hidden tiles
FT = F // P        # 32 f tiles
NCH = H // 512     # 4 chunks of 512

# ------------------------------------------------- tail-drain walrus patch
# The walrus in this container accepts at most ONE sync wait per Drain
# instruction; Tile's kernel-tail drain aggregates every outstanding sem
# into one Drain. Split it into a chain of single-wait drains.
_MAXW = 1


def _patched_drain_and_barrier(self, tick_clock, wait_clock):
    drain_inst = self.nc.sync.drain()
    wait_clock.add_sem_waits(
        drain_inst.ins, ScopedClock({None: tick_clock.global_clock})
    )
    si = drain_inst.ins.sync_info
    if si is not None and si.on_wait and len(si.on_wait) > _MAXW:
        waits = list(si.on_wait)
        si.on_wait = waits[:_MAXW]
        rest = waits[_MAXW:]
        while rest:
            d2 = self.nc.sync.drain()
            chunk, rest = rest[:_MAXW], rest[_MAXW:]
            s2 = d2.ins.sync_info
            if s2 is None:
                d2.ins.sync_info = mybir.SyncInfo(on_wait=chunk, on_update=[])
            else:
                s2.on_wait = chunk
    self.nc.all_engine_barrier()
    assert self.sems is not None
    popped = self.nc._tile_sem_poison_stack.pop()
    assert popped is self._sem_poison
    self.nc.clear_and_free_semaphores(list(self.sems.allocated().values()))
    self.nc.all_engine_barrier()


tile.TileContext._drain_and_barrier = _patched_drain_and_barrier





def build():
    nc = bass.Bass("TRN2", target_bir_lowering=False, debug=False,
                   num_devices=NCORES)

    # ------------------------------------------------------------- I/O
    x_full = nc.dram_tensor("x_full", [S, H], F32, kind="ExternalInput")
    xq = nc.dram_tensor("xq", [QR, H], F32, kind="ExternalInput")
    wq_s = nc.dram_tensor("wq_s", [H, NH * HD], F32R, kind="ExternalInput")
    wk_s = nc.dram_tensor("wk_s", [H, NKV * HD], F32R, kind="ExternalInput")
    wv_s = nc.dram_tensor("wv_s", [H, NKV * HD], F32R, kind="ExternalInput")
    wo = nc.dram_tensor("wo", [NH * HD, H], F32R, kind="ExternalInput")
    cosF = nc.dram_tensor("cosF", [HD, S], F32, kind="ExternalInput")
    sinF = nc.dram_tensor("sinF", [HD, S], F32, kind="ExternalInput")
    cosqF = nc.dram_tensor("cosqF", [HD, QR], F32, kind="ExternalInput")
    sinqF = nc.dram_tensor("sinqF", [HD, QR], F32, kind="ExternalInput")
    permT = nc.dram_tensor("permT", [HD, HD], F32R, kind="ExternalInput")
    maskb = nc.dram_tensor("maskb", [QR, S], mybir.dt.bfloat16, kind="ExternalInput")
    wg_s = nc.dram_tensor("wg_s", [H, E], F32, kind="ExternalInput")
    esel = nc.dram_tensor("esel", [P, E], F32, kind="ExternalInput")
    w1_s = nc.dram_tensor("w1_s", [H, F], F32R, kind="ExternalInput")
    w3_s = nc.dram_tensor("w3_s", [H, F], F32R, kind="ExternalInput")
    w2e = nc.dram_tensor("w2e", [F, H], F32R, kind="ExternalInput")

    resid_out = nc.dram_tensor("resid_out", [QR, H], F32, kind="ExternalOutput")
    moe_out = nc.dram_tensor("moe_out", [QR, H], F32, kind="ExternalOutput")

    with tile.TileContext(nc) as tc:
        const = tc.alloc_tile_pool(name="const", bufs=1)
        ident = const.tile([P, P], F32)
        make_identity(nc, ident)
        esel_sb = const.tile([P, E], F32)
        nc.sync.dma_start(esel_sb[:], esel[:, :])
        eps_sb = const.tile([P, 1], F32)
        nc.vector.memset(eps_sb[:], EPS)

        dram = tc.alloc_tile_pool(name="dram", bufs=1, space="DRAM")
        h2_bounce = dram.tile([QR, H], F32)
        h2_full = dram.tile([S, H], F32, addr_space="Shared")
        gt_buf = dram.tile([F, S], F32R)
        moe_part = dram.tile([S, H], F32)
        moe_rs = dram.tile([QR, H], F32)

        def _transp(pst, dst_ap, src_ap):
            pt = pst.tile([P, P], F32, tag="ps_tr")
            nc.tensor.transpose(pt[:], src_ap, ident[:])
            nc.vector.tensor_copy(dst_ap, pt[:])

        def rmsnorm_rows(pool, xt, width):
            """xt [128, width] -> x * rsqrt(mean(x^2)+eps), in place."""
            sq = pool.tile([P, width], F32, tag="rms_sq")
            ssum = pool.tile([P, 1], F32, tag="rms_ss")
            nc.scalar.activation(sq[:], xt, ACT.Square, accum_out=ssum[:])
            std = pool.tile([P, 1], F32, tag="rms_std")
            nc.scalar.activation(std[:], ssum[:], ACT.Sqrt, bias=eps_sb[:],
                                 scale=1.0 / width)
            rs = pool.tile([P, 1], F32, tag="rms_rs")
            nc.vector.reciprocal(rs[:], std[:])
            nc.vector.tensor_scalar_mul(xt, xt, rs[:])

        def rope(pool, psp, perm_sb, t_ap, cos_ap, sin_ap, width):
            """in-place neox rope on t_ap [128, width] (head-dim major).
            rot = P @ t (PE matmul with +-1 perm matrix), out = t*cos + rot*sin.
            """
            rp = psp.tile([P, width], F32, tag="rope_ps")
            nc.tensor.matmul(rp[:], perm_sb, t_ap, start=True, stop=True)
            a = pool.tile([P, width], F32, tag="rope_a")
            nc.vector.tensor_mul(a[:], t_ap, cos_ap)
            b = pool.tile([P, width], F32, tag="rope_b")
            nc.vector.tensor_mul(b[:], rp[:], sin_ap)
            nc.vector.tensor_add(t_ap, a[:], b[:])

        # persistent attention SBUF (freed after S2)
        attn = tc.alloc_tile_pool(name="attn", bufs=1)
        KTs = [attn.tile([P, S], F32R, tag=f"kt{h}", name=f"kt{h}") for h in range(NKV)]
        Vh = [[attn.tile([P, P], F32R, tag=f"v{h}_{k}", name=f"v{h}_{k}") for k in range(TT)]
              for h in range(NKV)]
        QTs = [attn.tile([P, QR], F32R, tag=f"qt{h}", name=f"qt{h}") for h in range(NH)]

        # ================================================= S1: projections
        with tc.tile_pool(name="s1", bufs=2) as s1, \
             tc.tile_pool(name="s1h", bufs=1) as s1h, \
             tc.tile_pool(name="s1w", bufs=4) as s1w, \
             tc.tile_pool(name="ps_t", bufs=2, space="PSUM") as pst, \
             tc.tile_pool(name="ps_rope", bufs=2, space="PSUM") as psr, \
             tc.tile_pool(name="ps_k", bufs=4, space="PSUM") as psk:
            cos_sb = s1h.tile([HD, S], F32, tag="cos")
            sin_sb = s1h.tile([HD, S], F32, tag="sin")
            nc.sync.dma_start(cos_sb[:], cosF[:, :])
            nc.sync.dma_start(sin_sb[:], sinF[:, :])
            cosq_sb = s1h.tile([HD, QR], F32, tag="cosq")
            sinq_sb = s1h.tile([HD, QR], F32, tag="sinq")
            nc.sync.dma_start(cosq_sb[:], cosqF[:, :])
            nc.sync.dma_start(sinq_sb[:], sinqF[:, :])
            perm_sb = s1h.tile([HD, HD], F32R, tag="perm")
            nc.sync.dma_start(perm_sb[:], permT[:, :])

            CW = 256
            for tch in range(S // CW):
                c0 = tch * CW
                hTc = s1h.tile([P, HT * CW], F32R, tag="hTc")
                for tt in range(CW // P):
                    xt = s1.tile([P, H], F32, tag="xt")
                    nc.sync.dma_start(
                        xt[:], x_full[c0 + tt * P:c0 + (tt + 1) * P, :])
                    rmsnorm_rows(s1, xt[:], H)
                    for j in range(HT):
                        _transp(pst,
                                hTc[:, j * CW + tt * P:j * CW + (tt + 1) * P],
                                xt[:, j * P:(j + 1) * P])
                for h in range(NKV):
                    for which, wsrc in ((0, wk_s), (1, wv_s)):
                        ps = psk.tile([P, 512], F32, tag="pskv")
                        for j in range(HT):
                            wt = s1w.tile([P, P], F32R, tag="wkv")
                            nc.sync.dma_start(
                                wt[:], wsrc[j * P:(j + 1) * P,
                                            h * HD:(h + 1) * HD])
                            nc.tensor.matmul(
                                ps[:, 0:CW], wt[:],
                                hTc[:, j * CW:(j + 1) * CW],
                                start=(j == 0), stop=(j == HT - 1))
                        if which == 0:
                            nc.vector.tensor_copy(KTs[h][:, c0:c0 + CW],
                                                  ps[:, 0:CW])
                        else:
                            vt = s1.tile([P, CW], F32, tag="vt")
                            nc.vector.tensor_copy(vt[:], ps[:, 0:CW])
                            for tt in range(CW // P):
                                _transp(pst, Vh[h][tch * (CW // P) + tt][:],
                                        vt[:, tt * P:(tt + 1) * P])
            for h in range(NKV):
                for ch in range(S // 512):
                    rope(s1, psr, perm_sb[:],
                         KTs[h][:, ch * 512:(ch + 1) * 512],
                         cos_sb[:, ch * 512:(ch + 1) * 512],
                         sin_sb[:, ch * 512:(ch + 1) * 512], 512)

            # Q for this core's rows
            hTq = s1h.tile([P, HT * QR], F32R, tag="hTq")
            for tt in range(QR // P):
                xt = s1.tile([P, H], F32, tag="xt")
                nc.sync.dma_start(xt[:], xq[tt * P:(tt + 1) * P, :])
                rmsnorm_rows(s1, xt[:], H)
                for j in range(HT):
                    _transp(pst,
                            hTq[:, j * QR + tt * P:j * QR + (tt + 1) * P],
                            xt[:, j * P:(j + 1) * P])
            for h in range(NH):
                ps = psk.tile([P, QR], F32, tag="pskv")
                for j in range(HT):
                    wt = s1w.tile([P, P], F32R, tag="wkv")
                    nc.sync.dma_start(
                        wt[:], wq_s[j * P:(j + 1) * P, h * HD:(h + 1) * HD])
                    nc.tensor.matmul(ps[:], wt[:],
                                     hTq[:, j * QR:(j + 1) * QR],
                                     start=(j == 0), stop=(j == HT - 1))
                nc.vector.tensor_copy(QTs[h][:], ps[:])
                rope(s1, psr, perm_sb[:], QTs[h][:], cosq_sb[:], sinq_sb[:], QR)

        # ============================================== S2: attention
        with tc.tile_pool(name="s2", bufs=2) as s2, \
             tc.tile_pool(name="s2s", bufs=3) as s2s, \
             tc.tile_pool(name="s2o", bufs=1) as s2o, \
             tc.tile_pool(name="ps_s", bufs=4, space="PSUM") as pss, \
             tc.tile_pool(name="ps_t2", bufs=2, space="PSUM") as pst2, \
             tc.tile_pool(name="ps_av", bufs=2, space="PSUM") as psav:
            OTs = [s2o.tile([P, QR], F32R, tag=f"ot{h}", name=f"ot{h}") for h in range(NH)]
            mbs = [s2o.tile([P, S], mybir.dt.bfloat16, tag=f"mb{qi}", name=f"mb{qi}")
                   for qi in range(QR // P)]
            for qi in range(QR // P):
                nc.sync.dma_start(mbs[qi][:], maskb[qi * P:(qi + 1) * P, :])
            for h in range(NH):
                kv = h // (NH // NKV)
                Pm = s2.tile([P, 2 * S], F32, tag="pmat")
                for qi in range(QR // P):
                    for ch in range(S // 512):
                        ps = pss.tile([P, 512], F32, tag="ps_s")
                        nc.tensor.matmul(
                            ps[:], QTs[h][:, qi * P:(qi + 1) * P],
                            KTs[kv][:, ch * 512:(ch + 1) * 512],
                            start=True, stop=True)
                        nc.vector.tensor_add(
                            Pm[:, qi * S + ch * 512:qi * S + (ch + 1) * 512],
                            ps[:], mbs[qi][:, ch * 512:(ch + 1) * 512])
                    row = Pm[:, qi * S:(qi + 1) * S]
                    m = s2s.tile([P, 1], F32, tag="sm_m")
                    nc.vector.reduce_max(m[:], row, axis=AX)
                    negm = s2s.tile([P, 1], F32, tag="sm_nm")
                    nc.vector.tensor_scalar_mul(negm[:], m[:], -1.0)
                    lsum = s2s.tile([P, 1], F32, tag="sm_l")
                    nc.scalar.activation(row, row, ACT.Exp, bias=negm[:],
                                         accum_out=lsum[:])
                    rl = s2s.tile([P, 1], F32, tag="sm_rl")
                    nc.vector.reciprocal(rl[:], lsum[:])
                    nc.vector.tensor_scalar_mul(row, row, rl[:])
                psa = psav.tile([P, QR], F32, tag="psav")
                for k in range(TT):
                    PT = s2s.tile([P, QR], F32R, tag="ptt")
                    for qi in range(QR // P):
                        _transp(pst2, PT[:, qi * P:(qi + 1) * P],
                                Pm[:, qi * S + k * P:qi * S + (k + 1) * P])
                    nc.tensor.matmul(psa[:], Vh[kv][k][:], PT[:],
                                     start=(k == 0), stop=(k == TT - 1))
                nc.vector.tensor_copy(OTs[h][:], psa[:])

            for qi in range(QR // P):
                resid_row = s2s.tile([P, H], F32, tag="rrow", bufs=2)
                for ch in range(NCH):
                    ps = pss.tile([P, 512], F32, tag="ps_s")
                    for h in range(NH):
                        wt = s2s.tile([P, 512], F32R, tag="wo_t")
                        nc.sync.dma_start(
                            wt[:], wo[h * HD:(h + 1) * HD,
                                      ch * 512:(ch + 1) * 512])
                        nc.tensor.matmul(
                            ps[:], OTs[h][:, qi * P:(qi + 1) * P],
                            wt[:], start=(h == 0), stop=(h == NH - 1))
                    xt = s2s.tile([P, 512], F32, tag="xt2")
                    nc.sync.dma_start(xt[:], xq[qi * P:(qi + 1) * P,
                                                ch * 512:(ch + 1) * 512])
                    nc.vector.tensor_add(xt[:], xt[:], ps[:])
                    nc.sync.dma_start(resid_out[qi * P:(qi + 1) * P,
                                                ch * 512:(ch + 1) * 512],
                                      xt[:])
                    nc.vector.tensor_copy(
                        resid_row[:, ch * 512:(ch + 1) * 512], xt[:])
                rmsnorm_rows(s2s, resid_row[:], H)
                nc.sync.dma_start(h2_bounce[qi * P:(qi + 1) * P, :],
                                  resid_row[:])

        attn.release()

        # ============================================== S3: AllGather h2
        nc.gpsimd.collective_compute(
            "AllGather", ALU.bypass,
            replica_groups=[list(range(NCORES))],
            ins=[h2_bounce[:].opt()], outs=[h2_full[:].opt()])

        # ============================================== S4: h2T + gating
        cvp = tc.alloc_tile_pool(name="cv", bufs=1)
        cvec = [cvp.tile([P, 1], F32, tag=f"cv{t}", name=f"cv{t}") for t in range(TT)]
        h2tp = tc.alloc_tile_pool(name="h2t", bufs=1)
        h2T = [h2tp.tile([P, S], F32R, tag=f"h2t{j}", name=f"h2t{j}") for j in range(HT)]
        with tc.tile_pool(name="s4", bufs=3) as s4, \
             tc.tile_pool(name="s4g", bufs=18) as s4g, \
             tc.tile_pool(name="ps_t4", bufs=4, space="PSUM") as pst4, \
             tc.tile_pool(name="ps_g", bufs=2, space="PSUM") as psg:
            wgt = s4.tile([P, HT * E], F32, tag="wg")
            for j in range(HT):
                nc.sync.dma_start(wgt[:, j * E:(j + 1) * E],
                                  wg_s[j * P:(j + 1) * P, :])
            for t in range(TT):
                xt = s4.tile([P, H], F32, tag="h2row")
                nc.sync.dma_start(xt[:], h2_full[t * P:(t + 1) * P, :])
                g32 = [s4g.tile([P, P], F32, tag="g32", name="g32")
                       for _ in range(HT)]
                for j in range(HT):
                    pt = pst4.tile([P, P], F32, tag="ps_tr4")
                    nc.tensor.transpose(pt[:], xt[:, j * P:(j + 1) * P],
                                        ident[:])
                    nc.vector.tensor_copy(h2T[j][:, t * P:(t + 1) * P], pt[:])
                    nc.vector.tensor_copy(g32[j][:], pt[:])
                ps = psg.tile([P, E], F32, tag="psgate")
                for j in range(HT):
                    nc.tensor.matmul(ps[:], g32[j][:],
                                     wgt[:, j * E:(j + 1) * E],
                                     start=(j == 0), stop=(j == HT - 1))
                lg = s4.tile([P, E], F32, tag="lg")
                m = s4.tile([P, 1], F32, tag="g_m")
                nc.vector.reduce_max(m[:], ps[:], axis=AX)
                negm = s4.tile([P, 1], F32, tag="g_nm")
                nc.vector.tensor_scalar_mul(negm[:], m[:], -1.0)
                se = s4.tile([P, 1], F32, tag="g_se")
                nc.scalar.activation(lg[:], ps[:], ACT.Exp, bias=negm[:],
                                     accum_out=se[:])
                rse = s4.tile([P, 1], F32, tag="g_rse")
                nc.vector.reciprocal(rse[:], se[:])
                nc.vector.tensor_scalar_mul(lg[:], lg[:], rse[:])
                m1 = s4.tile([P, 1], F32, tag="g_m1")
                nc.vector.reduce_max(m1[:], lg[:], axis=AX)
                top1 = s4.tile([P, E], F32, tag="g_t1")
                nc.vector.tensor_scalar(top1[:], lg[:], m1[:], None,
                                        op0=ALU.is_ge)
                big = s4.tile([P, E], F32, tag="g_big")
                nc.vector.tensor_scalar_mul(big[:], top1[:], 1e30)
                pm = s4.tile([P, E], F32, tag="g_pm")
                nc.vector.tensor_sub(pm[:], lg[:], big[:])
                m2 = s4.tile([P, 1], F32, tag="g_m2")
                nc.vector.reduce_max(m2[:], pm[:], axis=AX)
                sel = s4.tile([P, E], F32, tag="g_sel")
                nc.vector.tensor_scalar(sel[:], lg[:], m2[:], None,
                                        op0=ALU.is_ge)
                wsum = s4.tile([P, 1], F32, tag="g_ws")
                nc.vector.tensor_add(wsum[:], m1[:], m2[:])
                rws = s4.tile([P, 1], F32, tag="g_rws")
                nc.vector.reciprocal(rws[:], wsum[:])
                cw = s4.tile([P, E], F32, tag="g_cw")
                nc.vector.tensor_mul(cw[:], lg[:], sel[:])
                nc.vector.tensor_scalar_mul(cw[:], cw[:], rws[:])
                nc.vector.tensor_mul(cw[:], cw[:], esel_sb[:])
                nc.vector.reduce_sum(cvec[t][:], cw[:], axis=AX)

        # ============================================== S5: MoE up (w1/w3)
        with tc.tile_pool(name="s5w", bufs=18) as s5w, \
             tc.tile_pool(name="s5", bufs=3) as s5, \
             tc.tile_pool(name="ps_a", bufs=4, space="PSUM") as psa5:
            for f in range(FT):
                w1t = [s5w.tile([P, P], F32R, tag="w1t", name="w1t") for _ in range(HT)]
                w3t = [s5w.tile([P, P], F32R, tag="w3t", name="w3t") for _ in range(HT)]
                for j in range(HT):
                    nc.sync.dma_start(w1t[j][:],
                                      w1_s[j * P:(j + 1) * P,
                                           f * P:(f + 1) * P])
                    nc.sync.dma_start(w3t[j][:],
                                      w3_s[j * P:(j + 1) * P,
                                           f * P:(f + 1) * P])
                for tch in range(S // 512):
                    pa = psa5.tile([P, 512], F32, tag="pA")
                    pb = psa5.tile([P, 512], F32, tag="pB")
                    for j in range(HT):
                        nc.tensor.matmul(
                            pa[:], w1t[j][:],
                            h2T[j][:, tch * 512:(tch + 1) * 512],
                            start=(j == 0), stop=(j == HT - 1))
                    for j in range(HT):
                        nc.tensor.matmul(
                            pb[:], w3t[j][:],
                            h2T[j][:, tch * 512:(tch + 1) * 512],
                            start=(j == 0), stop=(j == HT - 1))
                    sil = s5.tile([P, 512], F32, tag="sil")
                    nc.scalar.activation(sil[:], pa[:], ACT.Silu)
                    gt = s5.tile([P, 512], F32R, tag="gt")
                    nc.vector.tensor_mul(gt[:], sil[:], pb[:])
                    nc.sync.dma_start(
                        gt_buf[f * P:(f + 1) * P,
                               tch * 512:(tch + 1) * 512], gt[:])

        h2tp.release()

        # ============================================== S6: MoE down (w2)
        accp = tc.alloc_tile_pool(name="acc", bufs=1)
        out_acc = [accp.tile([P, H], F32, tag=f"oa{t}", name=f"oa{t}") for t in range(TT)]
        with tc.tile_pool(name="s6w", bufs=5) as s6w, \
             tc.tile_pool(name="s6g", bufs=6) as s6g, \
             tc.tile_pool(name="ps_b", bufs=8, space="PSUM") as psb:
            NG = 4
            for g in range(FT // NG):
                w2g = [s6w.tile([P, H], F32R, tag="w2g", name="w2g") for _ in range(NG)]
                for fi in range(NG):
                    f = g * NG + fi
                    nc.sync.dma_start(w2g[fi][:], w2e[f * P:(f + 1) * P, :])
                for t in range(TT):
                    gtt = [s6g.tile([P, P], F32R, tag="gtt", name="gtt")
                           for _ in range(NG)]
                    for fi in range(NG):
                        f = g * NG + fi
                        nc.sync.dma_start(gtt[fi][:],
                                          gt_buf[f * P:(f + 1) * P,
                                                 t * P:(t + 1) * P])
                    for ch in range(NCH):
                        ps = psb.tile([P, 512], F32, tag="psb")
                        for fi in range(NG):
                            nc.tensor.matmul(
                                ps[:], gtt[fi][:],
                                w2g[fi][:, ch * 512:(ch + 1) * 512],
                                start=(fi == 0), stop=(fi == NG - 1))
                        if g == 0:
                            nc.vector.tensor_copy(
                                out_acc[t][:, ch * 512:(ch + 1) * 512], ps[:])
                        else:
                            nc.vector.tensor_add(
                                out_acc[t][:, ch * 512:(ch + 1) * 512],
                                out_acc[t][:, ch * 512:(ch + 1) * 512],
                                ps[:])
            for t in range(TT):
                nc.vector.tensor_scalar_mul(out_acc[t][:], out_acc[t][:],
                                            cvec[t][:])
                nc.sync.dma_start(moe_part[t * P:(t + 1) * P, :],
                                  out_acc[t][:])
        accp.release()
        cvp.release()

        # ============================================== S7: ReduceScatter
        nc.gpsimd.collective_compute(
            "ReduceScatter", ALU.add,
            replica_groups=[list(range(NCORES))],
            ins=[moe_part[:].opt()], outs=[moe_rs[:].opt()])
        with tc.tile_pool(name="s7", bufs=2) as s7:
            for qi in range(QR // P):
                ot = s7.tile([P, H], F32, tag="ot7")
                nc.sync.dma_start(ot[:], moe_rs[qi * P:(qi + 1) * P, :])
                nc.sync.dma_start(moe_out[qi * P:(qi + 1) * P, :], ot[:])

        dram.release()
        const.release()

    _split_excess_waits(nc)
    return nc


def _split_excess_waits(nc, maxw=1):
    """walrus in this container allows at most 2 sync waits per instruction;
    move excess waits onto same-engine NoOps inserted just before."""
    import copy as _copy
    templates = {}
    cur = nc.cur_bb.bb
    for eng in ("scalar", "vector", "tensor", "gpsimd", "sync"):
        bi = getattr(nc, eng).nop()
        templates[bi.ins.engine] = bi.ins
    for t in templates.values():
        cur.instructions.remove(t)
    k = 0
    for fn in nc.m.functions:
        for blk in fn.blocks:
            newlist = []
            changed = False
            for ins in blk.instructions:
                si = ins.sync_info
                waits = list(si.on_wait) if (si is not None and si.on_wait) else []
                if len(waits) > maxw:
                    changed = True
                    si.on_wait = waits[:maxw]
                    extra = waits[maxw:]
                    tpl = templates.get(ins.engine)
                    assert tpl is not None, f"no nop template for {ins.engine}"
                    while extra:
                        chunk, extra = extra[:maxw], extra[maxw:]
                        n2 = _copy.copy(tpl)
                        k += 1
                        n2.name = f"I-nopw{k}"
                        n2.sync_info = mybir.SyncInfo(on_wait=chunk,
                                                      on_update=[])
                        nc.register_instruction(n2)
                        newlist.append(n2)
                    newlist.append(ins)
                else:
                    newlist.append(ins)
            if changed:
                blk.instructions[:] = newlist


_NC_CACHE = None


def _get_nc():
    global _NC_CACHE
    if _NC_CACHE is None:
        _NC_CACHE = build()
    return _NC_CACHE


def _prep_inputs(inputs):
    pos = np.asarray(inputs["positions"])
    x = np.asarray(inputs["hidden_states"], dtype=np.float32).reshape(S, H)
    wq = np.asarray(inputs["wq"], dtype=np.float32)
    wk = np.asarray(inputs["wk"], dtype=np.float32)
    wv = np.asarray(inputs["wv"], dtype=np.float32)
    wo_ = np.asarray(inputs["wo"], dtype=np.float32)
    wg = np.asarray(inputs["w_gate"], dtype=np.float32)
    w1 = np.asarray(inputs["w1"], dtype=np.float32)
    w2 = np.asarray(inputs["w2"], dtype=np.float32)
    w3 = np.asarray(inputs["w3"], dtype=np.float32)
    ln_in = np.asarray(inputs["ln_in"], dtype=np.float32)
    ln_post = np.asarray(inputs["ln_post"], dtype=np.float32)

    half = HD // 2
    inv_freq = 1.0 / (THETA ** (np.arange(half, dtype=np.float32) * 2.0 / HD))
    ang = pos.astype(np.float32)[:, None] * inv_freq[None, :]   # [S, half]
    cosT_np = np.ascontiguousarray(np.cos(ang).T.astype(np.float32))
    sinT_np = np.ascontiguousarray(np.sin(ang).T.astype(np.float32))
    cosF_np = np.ascontiguousarray(np.concatenate([cosT_np, cosT_np], 0))
    sinF_np = np.ascontiguousarray(np.concatenate([sinT_np, sinT_np], 0))
    # rot[i] = -x[i+64] (i<64), +x[i-64] (i>=64); lhsT = Perm^T
    permM = np.zeros((HD, HD), dtype=np.float32)
    for i in range(half):
        permM[i, i + half] = -1.0
        permM[i + half, i] = 1.0
    permT_np = np.ascontiguousarray(permM.T)

    wq_sn = np.ascontiguousarray((wq * ln_in[:, None]) * (HD ** -0.5))
    wk_sn = np.ascontiguousarray(wk * ln_in[:, None])
    wv_sn = np.ascontiguousarray(wv * ln_in[:, None])
    wg_sn = np.ascontiguousarray(wg * ln_post[:, None])

    kcol = np.arange(S)
    in_maps = []
    for c in range(NCORES):
        r0 = c * QR
        qrow = r0 + np.arange(QR)
        import ml_dtypes
        mb = np.where(kcol[None, :] <= qrow[:, None], 0.0,
                      NEG).astype(ml_dtypes.bfloat16)
        es = np.zeros((P, E), dtype=np.float32)
        es[:, c] = 1.0
        in_maps.append({
            "x_full": x,
            "xq": np.ascontiguousarray(x[r0:r0 + QR]),
            "wq_s": wq_sn,
            "wk_s": wk_sn,
            "wv_s": wv_sn,
            "wo": wo_,
            "cosF": cosF_np,
            "sinF": sinF_np,
            "cosqF": np.ascontiguousarray(cosF_np[:, r0:r0 + QR]),
            "sinqF": np.ascontiguousarray(sinF_np[:, r0:r0 + QR]),
            "permT": permT_np,
            "maskb": mb,
            "wg_s": wg_sn,
            "esel": es,
            "w1_s": np.ascontiguousarray(w1[c] * ln_post[:, None]),
            "w3_s": np.ascontiguousarray(w3[c] * ln_post[:, None]),
            "w2e": np.ascontiguousarray(w2[c]),
        })
    return in_maps


def kernel(**inputs):
    nc = _get_nc()
    in_maps = _prep_inputs(inputs)
    res = run_bass_kernel_spmd(nc, in_maps, core_ids=list(range(NCORES)))
    moe = np.concatenate([res.results[c]["moe_out"] for c in range(NCORES)], 0)
    resid = np.concatenate([res.results[c]["resid_out"]
                            for c in range(NCORES)], 0)
    return (moe.reshape(B, S, H).astype(np.float32),
            resid.reshape(B, S, H).astype(np.float32))

